# revision 18
# baseline (speedup 1.0000x reference)
"""FCCaps (EfficientCapsNet fully-connected capsule routing) Trainium2 kernel.

Problem:  U_l (64, 512, 16) f32, W (512, 32, 16, 32) f32
    U_hat = einsum('bij,ikjl->bikl', U_l, W)          (B, n_l, n_h, d_h)
    A     = einsum('bikl,bhkl->bhik', U_hat, U_hat)
    C     = softmax(sum_i A / sqrt(d_h), axis=-1)      (B, n_l, n_h)
    U_h   = einsum('bikl,bik->bkl', U_hat, C)          (B, n_h, d_h)
    out   = squash(U_h)

Key algebraic identity used:
    A_sum[b,h,k] = sum_l U_hat[b,h,k,l] * S[b,k,l],  S = sum_i U_hat
so the (B, n_l, n_l, n_h) agreement tensor is never materialized.

Distribution: shard n_l (the i axis) 8 ways.  Each core computes U_hat for its
64 i's and all 64 batches, a partial S (AllReduce, 256KB), local softmax
(k is fully local), partial U_h (ReduceScatter over batch), then squash of its
own 8-batch output slice.  Host concatenates the 8 slices.

Host-side prep is layout-only (transpose/reshape/zero-fill of inputs).
"""

import sys

sys.path.insert(0, "/opt/trn_rl_repo")

import numpy as np

import concourse.bass as bass
import concourse.mybir as mybir
import concourse.tile as tile
from concourse import bacc
from concourse.bass_utils import run_bass_kernel_spmd

F32 = mybir.dt.float32
AX = mybir.AxisListType
OP = mybir.AluOpType
AF = mybir.ActivationFunctionType

B, NL, NH, DL, DH = 64, 512, 32, 16, 32
NCORES = 8
ILOC = NL // NCORES          # 64 i's per core
NG = ILOC // 8               # 8 groups of 8 i_sub
NCB = B // 16                # 4 batch chunks of 16
KL = NH * DH                 # 1024
ATT = 5.656854249492381      # sqrt(d_h)
EPS = 1e-20

_CACHE = {}


def _bcast(ap, n):
    """Append a step-0 innermost dim (read-broadcast) to an AP."""
    return bass.AP(ap.tensor, ap.offset, list(ap.ap) + [[0, n]])


def build_program():
    nc = bacc.Bacc(
        "TRN2",
        target_bir_lowering=False,
        debug=False,
        enable_asserts=False,
        num_devices=NCORES,
    )

    # WUL[g] = concat(Wr[g] (1024 cols), UlT[g] (64), Lb[g,0..3] (4x128)) so one
    # DMA covers all matmul operands of a group (keeps <=1 sem wait per matmul;
    # walrus rejects LDWEIGHTS with 2 waits).
    WUL = nc.dram_tensor("WUL", [NG, 128, 1600], F32, kind="ExternalInput")
    Ones = nc.dram_tensor("Ones", [NCB, 128, B], F32, kind="ExternalInput")
    out_sh = nc.dram_tensor("out_sh", [B // NCORES, KL], F32, kind="ExternalOutput")

    S_part = nc.dram_tensor("S_part", [B, KL], F32)
    S_full = nc.dram_tensor("S_full", [B, KL], F32, addr_space="Shared")
    Uh_part = nc.dram_tensor("Uh_part", [B, KL], F32)
    Uh_my = nc.dram_tensor("Uh_my", [B // NCORES, KL], F32)

    rg = [list(range(NCORES))]

    from contextlib import ExitStack

    with tile.TileContext(nc) as tc, ExitStack() as ctx:
        # ---- persistent pools ----
        persist = ctx.enter_context(tc.tile_pool(name="persist", bufs=1))
        uhat = persist.tile([128, NG, NCB, KL], F32)       # 128KB/partition
        srep = persist.tile([128, NCB, KL], F32)           # S replicated per (i_sub,b)
        asum = persist.tile([128, NCB, NG, 32], F32)       # A_sum: [p,(cb,g,k)]
        cc = persist.tile([128, NCB, NG, 32], F32)         # softmax C
        ones_sb = persist.tile([128, NCB, B], F32)
        tmp_pool = ctx.enter_context(tc.tile_pool(name="tmp", bufs=2))
        small = ctx.enter_context(tc.tile_pool(name="small", bufs=1))
        ps_uh_pool = ctx.enter_context(
            tc.tile_pool(name="psuh", bufs=1, space="PSUM")
        )

        nc.sync.dma_start(
            out=ones_sb[:],
            in_=Ones[:, :, :].rearrange("c p b -> p c b"),
        )

        # warm the PE's view of ones_sb so later matmuls need no extra wait
        ps_uh = ps_uh_pool.tile([B, KL], F32)
        nc.tensor.matmul(
            ps_uh[0:1, 0:1],
            lhsT=ones_sb[:, 0, 0:1],
            rhs=ones_sb[:, 0, 0:1],
            start=True,
            stop=True,
        )

        # ================= phase A: U_hat + partial S =================
        with (
            tc.tile_pool(name="wul", bufs=2) as wul_pool,
            tc.tile_pool(name="psu", bufs=2, space="PSUM") as psu_pool,
            tc.tile_pool(name="pss", bufs=1, space="PSUM") as pss_pool,
        ):
            ps_s = pss_pool.tile([B, KL], F32)
            for g in range(NG):
                wul_g = wul_pool.tile([128, 1600], F32)
                nc.sync.dma_start(out=wul_g[:], in_=WUL[g])
                wr_g = wul_g[:, 0:KL]
                ult_g = wul_g[:, KL : KL + B]
                for nch in range(2):
                    nc.tensor.matmul(
                        ps_s[:, nch * 512 : (nch + 1) * 512],
                        lhsT=ult_g,
                        rhs=wr_g[:, nch * 512 : (nch + 1) * 512],
                        start=(g == 0),
                        stop=(g == NG - 1),
                    )
                for cb in range(NCB):
                    lb = wul_g[:, KL + B + 128 * cb : KL + B + 128 * (cb + 1)]
                    psu = psu_pool.tile([128, KL], F32)
                    for nch in range(2):
                        nc.tensor.matmul(
                            psu[:, nch * 512 : (nch + 1) * 512],
                            lhsT=lb,
                            rhs=wr_g[:, nch * 512 : (nch + 1) * 512],
                            start=True,
                            stop=True,
                        )
                    # alternate PSUM->SBUF copy engine to balance ACT/DVE
                    if (g * NCB + cb) % 2 == 0:
                        nc.scalar.copy(out=uhat[:, g, cb, :], in_=psu[:])
                    else:
                        nc.vector.tensor_copy(uhat[:, g, cb, :], psu[:])

            s_sb = small.tile([B, KL], F32)
            nc.scalar.copy(out=s_sb[:], in_=ps_s[:])
            nc.sync.dma_start(out=S_part[:, :], in_=s_sb[:])

        # ================= S AllReduce + replicate =================
        nc.gpsimd.collective_compute(
            "AllReduce",
            OP.add,
            replica_groups=rg,
            ins=[S_part[:, :]],
            outs=[S_full[:, :]],
        )
        # srep[(i_sub,br), cb, :] = S_full[16cb+br, :]
        s_view = S_full[:, :].rearrange("(cb br) n -> br cb n", cb=NCB)
        for i_sub in range(8):
            nc.sync.dma_start(
                out=srep[16 * i_sub : 16 * (i_sub + 1), :, :], in_=s_view
            )

        # ================= phase B: A_sum + softmax =================
        for cb in range(NCB):
            for g in range(NG):
                tmp = tmp_pool.tile([128, 32, 32], F32)
                eng = nc.vector if (g % 2 == 0) else nc.gpsimd
                eng.tensor_tensor(
                    tmp[:],
                    uhat[:, g, cb, :].rearrange("p (k l) -> p k l", l=32),
                    srep[:, cb, :].rearrange("p (k l) -> p k l", l=32),
                    OP.mult,
                )
                nc.vector.tensor_reduce(
                    asum[:, cb, g, :], tmp[:], axis=AX.X, op=OP.add
                )
            # softmax over k for this cb
            mx = small.tile([128, NG], F32)
            nc.vector.tensor_reduce(mx[:], asum[:, cb], axis=AX.X, op=OP.max)
            zs = small.tile([128, NG, 32], F32)
            nc.vector.tensor_tensor(
                zs[:], asum[:, cb], _bcast(mx[:], 32), OP.subtract
            )
            ex = small.tile([128, NG, 32], F32)
            nc.scalar.activation(ex[:], zs[:], AF.Exp, scale=1.0 / ATT)
            sm = small.tile([128, NG], F32)
            nc.vector.tensor_reduce(sm[:], ex[:], axis=AX.X, op=OP.add)
            rc = small.tile([128, NG], F32)
            nc.vector.reciprocal(rc[:], sm[:])
            nc.vector.tensor_tensor(cc[:, cb], ex[:], _bcast(rc[:], 32), OP.mult)

        # ================= phase C: U_h partial =================
        for cb in range(NCB):
            for g in range(NG):
                tmp2 = tmp_pool.tile([128, 32, 32], F32)
                eng = nc.vector if (g % 2 == 0) else nc.gpsimd
                eng.tensor_tensor(
                    tmp2[:],
                    uhat[:, g, cb, :].rearrange("p (k l) -> p k l", l=32),
                    _bcast(cc[:, cb, g, :], 32),
                    OP.mult,
                )
                for nch in range(2):
                    nc.tensor.matmul(
                        ps_uh[:, nch * 512 : (nch + 1) * 512],
                        lhsT=ones_sb[:, cb, :],
                        rhs=tmp2[:].rearrange("p a b -> p (a b)")[
                            :, nch * 512 : (nch + 1) * 512
                        ],
                        start=(g == 0 and cb == 0),
                        stop=(g == NG - 1 and cb == NCB - 1),
                    )

        uh_sb = small.tile([B, KL], F32)
        nc.scalar.copy(out=uh_sb[:], in_=ps_uh[:])
        nc.sync.dma_start(out=Uh_part[:, :], in_=uh_sb[:])

        # ================= phase D: ReduceScatter + squash =================
        nc.gpsimd.collective_compute(
            "ReduceScatter",
            OP.add,
            replica_groups=rg,
            ins=[Uh_part[:, :]],
            outs=[Uh_my[:, :]],
        )
        nb = B // NCORES  # 8
        um = small.tile([nb, NH, DH], F32)
        nc.sync.dma_start(
            out=um[:], in_=Uh_my[:, :].rearrange("b (k l) -> b k l", l=DH)
        )
        sq = small.tile([nb, NH, DH], F32)
        nc.vector.tensor_tensor(sq[:], um[:], um[:], OP.mult)
        n2 = small.tile([nb, NH], F32)
        nc.vector.tensor_reduce(n2[:], sq[:], axis=AX.X, op=OP.add)
        nrm = small.tile([nb, NH], F32)
        nc.scalar.activation(nrm[:], n2[:], AF.Sqrt)
        ncl = small.tile([nb, NH], F32)
        nc.vector.tensor_scalar_min(ncl[:], nrm[:], 60.0)
        en = small.tile([nb, NH], F32)
        nc.scalar.activation(en[:], ncl[:], AF.Exp)
        re = small.tile([nb, NH], F32)
        nc.vector.reciprocal(re[:], en[:])
        one_t = small.tile([nb, NH], F32)
        nc.vector.memset(one_t[:], 1.0)
        f1 = small.tile([nb, NH], F32)
        nc.vector.tensor_tensor(f1[:], one_t[:], re[:], OP.subtract)
        nd = small.tile([nb, NH], F32)
        nc.vector.tensor_scalar_add(nd[:], nrm[:], EPS)
        rn = small.tile([nb, NH], F32)
        nc.vector.reciprocal(rn[:], nd[:])
        fac = small.tile([nb, NH], F32)
        nc.vector.tensor_tensor(fac[:], f1[:], rn[:], OP.mult)
        ov = small.tile([nb, NH, DH], F32)
        nc.vector.tensor_tensor(ov[:], um[:], _bcast(fac[:], DH), OP.mult)
        nc.sync.dma_start(
            out=out_sh[:, :], in_=ov[:].rearrange("b k l -> b (k l)")
        )

    nc.finalize()
    return nc


def host_prep(U_l, W):
    """Layout-only preprocessing of the full inputs into per-core in_maps."""
    U_l = np.asarray(U_l, dtype=np.float32)
    W = np.asarray(W, dtype=np.float32)
    # Ones[cb, 16*i_sub+br, b'] = 1 iff b' == 16*cb + br  (partition-sum matrix)
    ones = np.zeros((NCB, 128, B), dtype=np.float32)
    for cb in range(NCB):
        for i_sub in range(8):
            ones[cb, 16 * i_sub : 16 * (i_sub + 1), 16 * cb : 16 * (cb + 1)] = np.eye(
                16, dtype=np.float32
            )
    in_maps = []
    for c in range(NCORES):
        i0 = c * ILOC
        Wsh = W[i0 : i0 + ILOC]                   # (64, 32, 16, 32)
        # Wr[g, 16*i_sub+j, 32*k+l] = W[i0+8g+i_sub, k, j, l]
        Wr = np.ascontiguousarray(
            Wsh.reshape(NG, 8, NH, DL, DH).transpose(0, 1, 3, 2, 4)
        ).reshape(NG, 128, KL)
        # UlT[g, 16*i_sub+j, b] = U_l[b, i0+8g+i_sub, j]
        Ush = U_l[:, i0 : i0 + ILOC, :]           # (64, 64, 16)
        UlT = np.ascontiguousarray(
            Ush.reshape(B, NG, 8, DL).transpose(1, 2, 3, 0)
        ).reshape(NG, 128, B)
        # Lb[g, cb, 16*i_sub+j, 16*i_sub+br] = U_l[16cb+br, i0+8g+i_sub, j]
        Lb = np.zeros((NG, NCB, 128, 128), dtype=np.float32)
        blocks = UlT.reshape(NG, 8, DL, NCB, 16)  # [g, i_sub, j, cb, br]
        for i_sub in range(8):
            Lb[:, :, 16 * i_sub : 16 * i_sub + DL, 16 * i_sub : 16 * (i_sub + 1)] = (
                blocks[:, i_sub].transpose(0, 2, 1, 3)
            )
        WUL = np.concatenate(
            [Wr, UlT, Lb.transpose(0, 2, 1, 3).reshape(NG, 128, NCB * 128)],
            axis=2,
        )
        in_maps.append({"WUL": WUL, "Ones": ones})
    return in_maps


def kernel(U_l, W, trace=False):
    if "nc" not in _CACHE:
        _CACHE["nc"] = build_program()
    nc = _CACHE["nc"]
    in_maps = host_prep(U_l, W)
    res = run_bass_kernel_spmd(
        nc, in_maps, core_ids=list(range(NCORES)), trace=trace
    )
    _CACHE["last_result"] = res
    out = np.concatenate(
        [res.results[c]["out_sh"].reshape(B // NCORES, NH, DH) for c in range(NCORES)],
        axis=0,
    )
    return out
